# revision 1
# baseline (speedup 1.0000x reference)
"""Trainium2 Bass kernel for nn_CombinedLoss (3-branch local NCC loss).

Design: shard D=160 across 8 cores (20 interior slices each, 5-voxel halo,
host zero-padded to [30, 202, 170] per core). Per core, two 128-row H tiles.
Per branch (raw / Laplacian-edge / Sobel-magnitude): build fields on
DVE/ACT, then for each of the 5 NCC fields (A, B, A2, B2, AB) run the
separable 9^3 box sum as: H-axis banded matmul on TensorE -> W-axis
cumsum (tensor_tensor_scan straight out of PSUM) + shifted subtract ->
D-axis cumsum + shifted subtract. NCC pointwise math via fused
scalar_tensor_tensor ops, reduced with accum_out. Host combines the
[128, 8] per-core partial sums into the scalar loss.
"""
import numpy as np

N_CORES = 8
D, H, W = 160, 192, 160
DS = D // N_CORES          # 20
PAD = 5
DP = DS + 2 * PAD          # 30
HP = H + 2 * PAD           # 202
WP = W + 2 * PAD           # 170
INV_WS = np.float32(1.0 / 729.0)
EPS = 1e-5
NVOX = float(D * H * W)

# (h0, acc_lo, acc_hi, rlo, rhi) per H tile phase
H_TILES = [(0, 5, 101, 5, 127), (74, 27, 123, 1, 123)]

_CACHE = {}


def _make_band(klo, khi):
    B = np.zeros((128, 128), np.float32)
    for r in range(128):
        for o in range(-4, 5):
            k = r + o
            if klo <= k < khi:
                B[k, r] = 1.0
    return B


def _build_program():
    import concourse.bass as bass
    import concourse.tile as tile
    from concourse import bacc, mybir

    f32 = mybir.dt.float32
    Alu = mybir.AluOpType
    nc = bacc.Bacc("TRN2", target_bir_lowering=False, debug=False,
                   num_devices=N_CORES)

    xt_d = nc.dram_tensor("xt", [DP, HP, WP], f32, kind="ExternalInput").ap()
    xp_d = nc.dram_tensor("xp", [DP, HP, WP], f32, kind="ExternalInput").ap()
    mk_d = nc.dram_tensor("mask", [128, 32], f32, kind="ExternalInput").ap()
    b0_d = nc.dram_tensor("band0", [128, 128], f32, kind="ExternalInput").ap()
    b1_d = nc.dram_tensor("band1", [128, 128], f32, kind="ExternalInput").ap()
    bl_d = nc.dram_tensor("band_lap", [128, 128], f32, kind="ExternalInput").ap()
    bs_d = nc.dram_tensor("band_121", [128, 128], f32, kind="ExternalInput").ap()
    bo_d = nc.dram_tensor("band_one", [128, 128], f32, kind="ExternalInput").ap()
    bd_d = nc.dram_tensor("band_drv", [128, 128], f32, kind="ExternalInput").ap()
    out_d = nc.dram_tensor("out", [128, 8], f32, kind="ExternalOutput").ap()

    with tile.TileContext(nc) as tc:
        with (
            tc.tile_pool(name="main", bufs=1) as pool,
            tc.tile_pool(name="psum", bufs=4, space="PSUM") as psum_pool,
        ):
            XT = pool.tile([128, DP * WP], f32, tag="XT")
            XP = pool.tile([128, DP * WP], f32, tag="XP")
            A = pool.tile([128, 29 * WP], f32, tag="A")
            B = pool.tile([128, 29 * WP], f32, tag="B")
            FT = pool.tile([128, 29 * WP], f32, tag="FT")     # A2/B2/AB + build scratch3
            T1 = pool.tile([128, 29 * WP], f32, tag="T1")     # tmp1 / csW
            T2 = pool.tile([128, 30 * 162], f32, tag="T2")    # tmp2 / R
            BF = [pool.tile([128, 160 * 20], f32, tag=f"BF{i}", name=f"BF{i}") for i in range(5)]
            MK = pool.tile([128, 32], f32, tag="MK")
            BAND = [pool.tile([128, 128], f32, tag=f"BAND{i}", name=f"BAND{i}") for i in range(6)]
            ACC = pool.tile([128, 8], f32, tag="ACC")

            def v3(t, d, w):   # [128, d, w] view
                return t[:].rearrange("p (d w) -> p d w", w=w)

            nc.sync.dma_start(MK[:], mk_d)
            for bt, bd in zip(BAND, [b0_d, b1_d, bl_d, bs_d, bo_d, bd_d]):
                nc.sync.dma_start(bt[:], bd)
            nc.vector.memset(A[:], 0.0)
            nc.vector.memset(B[:], 0.0)
            nc.vector.memset(ACC[:], 0.0)

            XT3 = v3(XT, DP, WP)
            XP3 = v3(XP, DP, WP)
            A3 = v3(A, 29, WP)
            B3 = v3(B, 29, WP)
            FT3 = v3(FT, 29, WP)
            T1b = T1[:, 0:30 * 162].rearrange("p (d w) -> p d w", w=162)
            T2b = v3(T2, 30, 162)
            FTb = FT[:, 0:30 * 162].rearrange("p (d w) -> p d w", w=162)
            CSW3 = v3(T1, 29, WP)

            def build_E(X3, dst3):
                nc.vector.tensor_add(T1b[:, 0:28, 0:160],
                                     X3[:, 1:29, 4:164], X3[:, 1:29, 6:166])
                nc.vector.tensor_add(T2b[:, 0:28, 0:160],
                                     X3[:, 0:28, 5:165], X3[:, 2:30, 5:165])
                nc.vector.tensor_add(FTb[:, 0:28, 0:160],
                                     T1b[:, 0:28, 0:160], T2b[:, 0:28, 0:160])
                nc.vector.scalar_tensor_tensor(
                    dst3[:, 1:29, 5:165], X3[:, 1:29, 5:165], 4.0,
                    FTb[:, 0:28, 0:160], Alu.mult, Alu.subtract)
                for d0 in range(1, 29, 3):
                    dc = min(3, 29 - d0)
                    ps = psum_pool.tile([128, 512], f32, tag="ps", name="ps")
                    nc.tensor.matmul(ps[:, 0:dc * WP], BAND[2][:],
                                     X3[:, d0:d0 + dc, :], start=True, stop=True)
                    ps3 = ps[:, 0:dc * WP].rearrange("p (d w) -> p d w", w=WP)
                    nc.vector.tensor_add(dst3[:, d0:d0 + dc, 5:165],
                                         dst3[:, d0:d0 + dc, 5:165],
                                         ps3[:, :, 5:165])

            def mask_field(dst3):
                for dpad in list(range(1, 5)) + list(range(25, 29)):
                    nc.vector.tensor_scalar_mul(
                        dst3[:, dpad:dpad + 1, 5:165],
                        dst3[:, dpad:dpad + 1, 5:165],
                        MK[:, dpad:dpad + 1])

            def g_mm_square(band_t, src3, dst3, first):
                # H-band matmul of src3 (d-idx 0:28 x w-idx 0:160), square the
                # PSUM result and write/accumulate into dst3[:, 1:29, 5:165]
                for c0 in range(0, 28, 3):
                    cc = min(3, 28 - c0)
                    ps = psum_pool.tile([128, 512], f32, tag="ps", name="ps")
                    nc.tensor.matmul(ps[:, 0:cc * 160], band_t[:],
                                     src3[:, c0:c0 + cc, 0:160],
                                     start=True, stop=True)
                    ps3 = ps[:, 0:cc * 160].rearrange("p (d w) -> p d w", w=160)
                    if first:
                        nc.scalar.square(dst3[:, 1 + c0:1 + c0 + cc, 5:165], ps3)
                    else:
                        nc.scalar.square(T2b[:, c0:c0 + cc, 0:160], ps3)
                        nc.vector.tensor_add(dst3[:, 1 + c0:1 + c0 + cc, 5:165],
                                             dst3[:, 1 + c0:1 + c0 + cc, 5:165],
                                             T2b[:, c0:c0 + cc, 0:160])

            def build_S(X3, dst3):
                # gx = s121H(onesD(derivW))
                nc.vector.tensor_sub(T1b[:, 0:30, 0:160],
                                     X3[:, 0:30, 6:166], X3[:, 0:30, 4:164])
                nc.vector.tensor_add(T2b[:, 0:28, 0:160],
                                     T1b[:, 0:28, 0:160], T1b[:, 2:30, 0:160])
                nc.vector.tensor_add(FTb[:, 0:28, 0:160],
                                     T2b[:, 0:28, 0:160], T1b[:, 1:29, 0:160])
                g_mm_square(BAND[3], FTb, dst3, True)
                # gy = ones3H(s121W(derivD))
                nc.vector.tensor_sub(T1b[:, 0:28, 0:162],
                                     X3[:, 2:30, 4:166], X3[:, 0:28, 4:166])
                nc.vector.tensor_add(T2b[:, 0:28, 0:160],
                                     T1b[:, 0:28, 0:160], T1b[:, 0:28, 2:162])
                nc.vector.scalar_tensor_tensor(
                    FTb[:, 0:28, 0:160], T1b[:, 0:28, 1:161], 2.0,
                    T2b[:, 0:28, 0:160], Alu.mult, Alu.add)
                g_mm_square(BAND[4], FTb, dst3, False)
                # gz = derivH(s121D(onesW))
                nc.vector.tensor_add(T1b[:, 0:30, 0:160],
                                     X3[:, 0:30, 4:164], X3[:, 0:30, 6:166])
                nc.vector.tensor_add(T2b[:, 0:30, 0:160],
                                     T1b[:, 0:30, 0:160], X3[:, 0:30, 5:165])
                nc.vector.tensor_add(FTb[:, 0:28, 0:160],
                                     T2b[:, 0:28, 0:160], T2b[:, 2:30, 0:160])
                nc.vector.scalar_tensor_tensor(
                    T1b[:, 0:28, 0:160], T2b[:, 1:29, 0:160], 2.0,
                    FTb[:, 0:28, 0:160], Alu.mult, Alu.add)
                g_mm_square(BAND[5], T1b, dst3, False)
                nc.scalar.sqrt(dst3[:, 1:29, 5:165], dst3[:, 1:29, 5:165])

            def box_pipe(F3, band_tile, bf):
                # H-band matmul in (d,w) chunks -> W cumsum from PSUM
                for d0 in range(0, 29, 3):
                    dc = min(3, 29 - d0)
                    ps = psum_pool.tile([128, 512], f32, tag="ps", name="ps")
                    nc.tensor.matmul(ps[:, 0:dc * WP], band_tile[:],
                                     F3[:, d0:d0 + dc, :], start=True, stop=True)
                    nc.vector.tensor_tensor_scan(
                        T1[:, d0 * WP:(d0 + dc) * WP], ps[:, 0:dc * WP],
                        XT[:, 0:dc * WP], 0.0, Alu.add, Alu.bypass)
                # W shifted-subtract, written d-minor into T2 (R)
                cswT = T1[:].rearrange("p (d w) -> p w d", w=WP)
                R3 = T2[:, 0:160 * 29].rearrange("p (w d) -> p w d", d=29)
                nc.vector.tensor_sub(R3, cswT[:, 9:169, :], cswT[:, 0:160, :])
                # D cumsum + shifted subtract
                nc.vector.tensor_tensor_scan(
                    T1[:, 0:160 * 29], T2[:, 0:160 * 29], T2[:, 0:160 * 29],
                    0.0, Alu.add, Alu.bypass)
                csd3 = T1[:, 0:160 * 29].rearrange("p (w d) -> p w d", d=29)
                bf3 = bf[:].rearrange("p (w k) -> p w k", k=20)
                nc.vector.tensor_sub(bf3, csd3[:, :, 9:29], csd3[:, :, 0:20])

            for ph, (h0, acc_lo, acc_hi, rlo, rhi) in enumerate(H_TILES):
                band = BAND[ph]
                nc.sync.dma_start(XT3, xt_d[:, h0:h0 + 128, :].transpose([1, 0, 2]))
                nc.sync.dma_start(XP3, xp_d[:, h0:h0 + 128, :].transpose([1, 0, 2]))

                for br in range(3):
                    if br == 0:
                        FA, FB = XT3[:, 0:29, :], XP3[:, 0:29, :]
                    elif br == 1:
                        build_E(XT3, A3)
                        mask_field(A3)
                        build_E(XP3, B3)
                        mask_field(B3)
                        FA, FB = A3, B3
                    else:
                        build_S(XT3, A3)
                        mask_field(A3)
                        build_S(XP3, B3)
                        mask_field(B3)
                        FA, FB = A3, B3

                    box_pipe(FA, band, BF[0])                   # Is
                    box_pipe(FB, band, BF[1])                   # Js
                    nc.scalar.square(FT3, FA)
                    box_pipe(FT3, band, BF[2])                  # I2s
                    nc.scalar.square(FT3, FB)
                    box_pipe(FT3, band, BF[3])                  # J2s
                    nc.vector.tensor_mul(FT3, FA, FB)
                    box_pipe(FT3, band, BF[4])                  # IJs

                    # NCC pointwise math on [128, 3200] box sums
                    N1, N2 = T1[:, 0:3200], T2[:, 0:3200]
                    Is, Js, I2s, J2s, IJs = (b[:] for b in BF)
                    stt = nc.vector.scalar_tensor_tensor
                    stt(N1, Is, float(INV_WS), Js, Alu.mult, Alu.mult)
                    nc.vector.tensor_sub(N2, IJs, N1)           # cross
                    stt(N1, Is, float(INV_WS), Is, Alu.mult, Alu.mult)
                    nc.vector.tensor_sub(Is, I2s, N1)           # Ivar -> BF0
                    stt(N1, Js, float(INV_WS), Js, Alu.mult, Alu.mult)
                    nc.vector.tensor_sub(Js, J2s, N1)           # Jvar -> BF1
                    stt(N1, Is, EPS, Js, Alu.add, Alu.mult)     # denom
                    nc.vector.reciprocal(I2s, N1)               # rden -> BF2
                    nc.scalar.square(N1, N2)                    # num = cross^2
                    stt(J2s, N1, 1.0, I2s, Alu.mult, Alu.mult,
                        accum_out=ACC[:, ph * 3 + br:ph * 3 + br + 1])

            nc.sync.dma_start(out_d, ACC[:])
    nc.compile()
    return nc


def _get_nc():
    if "nc" not in _CACHE:
        _CACHE["nc"] = _build_program()
    return _CACHE["nc"]


def _host_inputs(y_true, y_pred):
    xt = np.ascontiguousarray(np.asarray(y_true, np.float32).reshape(D, H, W))
    xp = np.ascontiguousarray(np.asarray(y_pred, np.float32).reshape(D, H, W))
    big_t = np.zeros((D + 2 * PAD, HP, WP), np.float32)
    big_p = np.zeros((D + 2 * PAD, HP, WP), np.float32)
    big_t[PAD:PAD + D, PAD:PAD + H, PAD:PAD + W] = xt
    big_p[PAD:PAD + D, PAD:PAD + H, PAD:PAD + W] = xp
    band0 = _make_band(5, 127)
    band1 = _make_band(1, 123)
    def bmat(taps):
        Bm = np.zeros((128, 128), np.float32)
        for o, t in taps:
            for r in range(128):
                if 0 <= r + o < 128:
                    Bm[r + o, r] += t
        return Bm
    band_lap = bmat([(-1, -1.0), (0, 2.0), (1, -1.0)])
    band_121 = bmat([(-1, 1.0), (0, 2.0), (1, 1.0)])
    band_one = bmat([(-1, 1.0), (0, 1.0), (1, 1.0)])
    band_drv = bmat([(-1, -1.0), (1, 1.0)])
    in_maps = []
    for c in range(N_CORES):
        d0 = c * DS
        mask = np.zeros((128, 32), np.float32)
        for j in range(DP):
            if 0 <= d0 - PAD + j < D:
                mask[:, j] = 1.0
        in_maps.append({
            "xt": np.ascontiguousarray(big_t[d0:d0 + DP]),
            "xp": np.ascontiguousarray(big_p[d0:d0 + DP]),
            "mask": mask,
            "band0": band0,
            "band1": band1,
            "band_lap": band_lap,
            "band_121": band_121,
            "band_one": band_one,
            "band_drv": band_drv,
        })
    return in_maps


def _combine(results):
    total = np.zeros(3, np.float64)
    for res in results:
        cols = np.asarray(res["out"], np.float64)
        for ph, (_, lo, hi, _, _) in enumerate(H_TILES):
            for br in range(3):
                total[br] += cols[lo:hi, ph * 3 + br].sum()
    losses = -total / NVOX
    return np.float32(0.8 * losses[0] + 0.1 * losses[1] + 0.1 * losses[2])


def kernel(y_true, y_pred):
    from concourse.bass_utils import run_bass_kernel_spmd
    nc = _get_nc()
    in_maps = _host_inputs(y_true, y_pred)
    res = run_bass_kernel_spmd(nc, in_maps, core_ids=list(range(N_CORES)))
    return _combine(res.results)


if __name__ == "__main__":
    g = np.load("/root/problem/golden.npz")
    got = float(kernel(g["y_true"], g["y_pred"]))
    exp = float(g["expected"])
    print(f"expected {exp:.9f} got {got:.9f} rel {abs(got-exp)/abs(exp):.3e}")

